# revision 5
# baseline (speedup 1.0000x reference)
"""Grouped GEMM (MoE expert-parallel) Bass kernel for Trainium2.

Problem: x (16384, 2048) fp32, weight (128*2048, 1408) fp32, batch_sizes (128,)
int32 summing to 16384 (tokens sorted by expert).
out[rows_e] = x[rows_e] @ W[e] for each expert e.

Strategy (expert-parallel across 8 NeuronCores):
  - 16 experts per core. Experts are sorted by batch size (descending) and
    dealt round-robin into 16 "slots" x 8 cores, so slot j holds experts of
    similar size on every core. Slot j gets a static token capacity
    cap_j = max over cores of bs (rounded up to 16), making the compiled
    program identical (SPMD) across cores while keeping padding tiny.
  - Host pre-transposes/pads x to xT (2048, T_pad) bf16 per core (resident
    in SBUF for the whole kernel), reorders weights to (16, 2048, 1408)
    bf16 per core. On-chip: out = xT.T @ w via TensorE with K=2048
    contracted in 16 chunks of 128 accumulating in PSUM; m-tiles of up to
    128 tokens (the last tile of a slot may be <128 partitions).
  - Output (T_pad, 1408) bf16 per core; host strips padding, upcasts to
    fp32, and scatters rows back.

Self-contained: needs only numpy/ml_dtypes + the concourse package.
"""

import os

import numpy as np
import ml_dtypes

import concourse.bass as bass
import concourse.mybir as mybir
import concourse.tile as tile
from concourse import bacc
from concourse.bass_utils import run_bass_kernel_spmd

E = 128          # num experts
M = 2048         # in features (contraction)
N = 1408         # out features
S = 16384        # tokens
NCORES = 8
EPC = E // NCORES      # experts per core = 16
KT = M // 128          # contraction tiles = 16
N_CHUNKS = [(0, 512), (512, 512), (1024, 384)]  # psum-bank-sized N tiles

BF16 = mybir.dt.bfloat16
FP32 = mybir.dt.float32

_program_cache: dict = {}
LAST_EXEC_NS = None
LAST_RESULTS = None


def _build_program(slot_caps):
    """Compile the SPMD Bass program for the given per-slot token caps."""
    slot_caps = [int(c) for c in slot_caps]
    T_pad = sum(slot_caps)
    slot_offs = np.concatenate([[0], np.cumsum(slot_caps)]).astype(int)
    nc = bacc.Bacc(
        "TRN2", target_bir_lowering=False, debug=False, num_devices=NCORES
    )
    # x pre-transposed + pre-swizzled on host: per slot a (128, KT*cap)
    # partition-major block (5KB contiguous per partition per slot DMA).
    xt_d = nc.dram_tensor("xt", [128, KT * T_pad], BF16, kind="ExternalInput").ap()
    w_d = nc.dram_tensor("w", [EPC, M, N], BF16, kind="ExternalInput").ap()
    out_d = nc.dram_tensor("out", [T_pad, N], BF16, kind="ExternalOutput").ap()

    # process slots interleaved big/small so per-slot PE time stays below
    # per-slot DMA time on average and buffers absorb the variance
    idx = [j for j in range(EPC) if slot_caps[j] > 0]
    order = []
    lo, hi = 0, len(idx) - 1
    while lo <= hi:
        order.append(idx[lo])
        if hi != lo:
            order.append(idx[hi])
        lo += 1
        hi -= 1

    with tile.TileContext(nc) as tc:
        with (
            tc.tile_pool(name="xp", bufs=3) as xp,
            tc.tile_pool(name="wp", bufs=3) as wp,
            tc.tile_pool(name="op", bufs=4) as op,
            tc.tile_pool(name="pp", bufs=2, space="PSUM") as pp,
        ):
            for j in order:
                cap = slot_caps[j]
                slot_off = int(slot_offs[j])
                # slot's xT block: (128, kt, cap) bf16, fully contiguous rows
                xt = xp.tile([128, KT, cap], BF16, tag="x", name=f"x{j}")
                nc.sync.dma_start(
                    xt[:],
                    xt_d[:, KT * slot_off : KT * (slot_off + cap)].rearrange(
                        "p (kt t) -> p kt t", kt=KT
                    ),
                )
                # whole expert weight, k-tiled: (128, kt, 1408) bf16,
                # loaded in two halves for smoother pipelining
                wt = wp.tile([128, KT, N], BF16, tag="w", name=f"w{j}")
                half = KT // 2
                wsrc = w_d[j].rearrange("(kt p) n -> kt p n", p=128)
                nc.sync.dma_start(
                    wt[:, 0:half, :],
                    wsrc[0:half].rearrange("kt p n -> p kt n"),
                )
                nc.sync.dma_start(
                    wt[:, half:KT, :],
                    wsrc[half:KT].rearrange("kt p n -> p kt n"),
                )
                m_off = 0
                while m_off < cap:
                    mr = min(128, cap - m_off)  # rows in this m-tile
                    ps = pp.tile([128, 3, 512], FP32, tag="ps", name=f"ps{j}_{m_off}")
                    for ni, (n0, nw) in enumerate(N_CHUNKS):
                        for k in range(KT):
                            nc.tensor.matmul(
                                ps[0:mr, ni, 0:nw],
                                xt[:, k, m_off : m_off + mr],
                                wt[:, k, n0 : n0 + nw],
                                start=(k == 0),
                                stop=(k == KT - 1),
                            )
                    ot = op.tile([128, N], BF16, tag="o", name=f"o{j}_{m_off}")
                    for ni, (n0, nw) in enumerate(N_CHUNKS):
                        nc.any.tensor_copy(ot[0:mr, n0 : n0 + nw], ps[0:mr, ni, 0:nw])
                    nc.sync.dma_start(
                        out_d[slot_off + m_off : slot_off + m_off + mr, :],
                        ot[0:mr, :],
                    )
                    m_off += mr
    nc.compile()
    return nc


def _plan(bs):
    """Assign experts to (core, slot) and compute slot capacities."""
    order = np.argsort(-bs, kind="stable")  # experts sorted desc by size
    # slot j on core c handles expert order[8*j + c]
    assign = order.reshape(EPC, NCORES)
    caps = bs[assign].max(axis=1)
    caps = ((caps + 15) // 16) * 16  # round to 16 for tidy strides
    return assign, caps.astype(np.int64)


def kernel(x: np.ndarray, weight: np.ndarray, batch_sizes: np.ndarray) -> np.ndarray:
    global LAST_EXEC_NS, LAST_RESULTS
    x = np.asarray(x)
    weight = np.asarray(weight)
    bs = np.asarray(batch_sizes).astype(np.int64)
    assert x.shape == (S, M) and weight.shape == (E * M, N)

    assign, caps = _plan(bs)
    T_pad = int(caps.sum())
    key = tuple(caps.tolist())
    if key not in _program_cache:
        _program_cache[key] = _build_program(caps)
    nc = _program_cache[key]

    offs = np.concatenate([[0], np.cumsum(bs)])
    slot_offs = np.concatenate([[0], np.cumsum(caps)])
    w3 = weight.reshape(E, M, N)

    xb = x.astype(ml_dtypes.bfloat16)
    in_maps = []
    for c in range(NCORES):
        # per slot: (128, KT, cap) partition-major block of xT
        xt_core = np.zeros((128, KT * T_pad), dtype=ml_dtypes.bfloat16)
        w_core = np.empty((EPC, M, N), dtype=ml_dtypes.bfloat16)
        for j in range(EPC):
            e = int(assign[j, c])
            b = int(bs[e])
            blk = np.zeros((KT, 128, int(caps[j])), dtype=ml_dtypes.bfloat16)
            # xT rows (M=KT*128) for this slot's tokens
            blk[:, :, :b] = (
                xb[offs[e] : offs[e] + b].T.reshape(KT, 128, b)
            )
            xt_core[:, KT * slot_offs[j] : KT * slot_offs[j + 1]] = (
                blk.transpose(1, 0, 2).reshape(128, -1)
            )
            w_core[j] = w3[e]
        in_maps.append({"xt": xt_core, "w": w_core})

    trace = os.environ.get("BASS_KERNEL_TRACE", "1") != "0"
    try:
        res = run_bass_kernel_spmd(
            nc, in_maps, core_ids=list(range(NCORES)), trace=trace
        )
    except ModuleNotFoundError:
        # NTFF profiling hook unavailable in this image — run untraced.
        res = run_bass_kernel_spmd(
            nc, in_maps, core_ids=list(range(NCORES)), trace=False
        )
    LAST_RESULTS = res
    LAST_EXEC_NS = res.exec_time_ns

    out = np.empty((S, N), dtype=np.float32)
    for c in range(NCORES):
        core_out = res.results[c]["out"]
        for j in range(EPC):
            e = int(assign[j, c])
            b = int(bs[e])
            out[offs[e] : offs[e] + b] = core_out[
                slot_offs[j] : slot_offs[j] + b
            ].astype(np.float32)
    return out
